# revision 12
# baseline (speedup 1.0000x reference)
"""Trainium2 Bass kernel for nn_Nibbler_70755291234540 (gnn_message_passing).

q = concat(obs, relu(per-gvf tiny nets(gathered obs))) @ q_W.T

Strategy (8 NeuronCores, SPMD single program):
  - Shard the 4096 GVFs across cores (512/core); every core sees the full
    batch and produces a partial Q; host sums the partials.
  - Host pre-transposes obs -> obsT (4096, 2048) fp16 in DRAM. The per-GVF
    input gather becomes a row gather out of obsT done by dma_gather
    (GPSIMD SWDGE) straight into partition-major SBUF tiles: 128 gathered
    rows = one group of 8 GVFs x 16 inputs; 8 groups (4 MB) per call, 8
    calls pipelined back-to-back at ~HBM line rate.
  - Per pair of groups: col-tiled (K=128, M=64)x2 fp16 matmuls run both
    groups concurrently in the 128x128 array; relu+cast eviction of the
    full 128-partition PSUM tile on ACT (half 0) / DVE (half 1).
  - Q head: 36 K-tiles (32 gvf-feature pairs + 4 raw-obs blocks) col-tiled
    4-wide at M=18 into one 128-partition PSUM accumulator per batch half;
    col groups are summed on the host together with the cross-core sum.
"""

import sys
import types

import numpy as np

# ---- problem constants (hardcoded; kernel.py must be self-contained) ----
B = 2048
OBS_DIM = 4096
N_GVFS = 4096
IPG = 16  # inputs per gvf
HPG = 8  # hidden per gvf
NA = 18  # actions
N_CORES = 8
GPC = N_GVFS // N_CORES  # 512 gvfs per core
N_GROUPS = GPC // 8  # 64 groups of 8 gvfs -> 128 gathered rows each
N_PAIRS = N_GROUPS // 2  # 32 pair tiles (128 feature rows each)
HALF = B // 2  # batch slab (PSUM-sized)
NB = 512  # matmul moving-operand chunk
CHUNK_GROUPS = 8  # gvf groups per dma_gather call (1024 rows, full batch)
N_CHUNKS = N_GROUPS // CHUNK_GROUPS  # 8
OWN_BLKS = (OBS_DIM // N_CORES) // 128  # 4 obs-feature blocks per core


def _install_axon_profile_hook():
    """bass_utils trace=True under axon needs antenv.axon_hooks; shim it."""
    try:
        import antenv
    except ImportError:
        return
    if "antenv.axon_hooks" in sys.modules:
        return
    hooks = types.ModuleType("antenv.axon_hooks")
    hooks._hook = None

    def set_axon_ntff_profile_hook(h):
        hooks._hook = h

    def get_axon_ntff_profile_hook():
        return hooks._hook

    hooks.set_axon_ntff_profile_hook = set_axon_ntff_profile_hook
    hooks.get_axon_ntff_profile_hook = get_axon_ntff_profile_hook
    sys.modules["antenv.axon_hooks"] = hooks
    antenv.axon_hooks = hooks
    try:
        from trn_agent_boot.trn_boot import _ntff_profile_via_ctypes

        hook = _ntff_profile_via_ctypes("/opt/axon/libaxon_pjrt.so")
        if hook is not None:
            set_axon_ntff_profile_hook(hook)
    except Exception:
        pass


_install_axon_profile_hook()

import concourse.bacc as bacc
import concourse.bass as bass
import concourse.mybir as mybir
import concourse.tile as tile
from concourse.bass_utils import run_bass_kernel_spmd

F16 = mybir.dt.float16
F32 = mybir.dt.float32
I16 = mybir.dt.int16

_PROGRAM = None


def _build_program():
    nc = bacc.Bacc(None, target_bir_lowering=False, debug=False, num_devices=N_CORES)

    obst = nc.dram_tensor("obst", [OBS_DIM, B], F16, kind="ExternalInput")
    # own obs block, pre-arranged (128, 4*B): col block ob holds obsT rows
    # [own0 + ob*128 + p]
    obst_own = nc.dram_tensor("obst_own", [128, OWN_BLKS * B], F16, kind="ExternalInput")
    wbd = nc.dram_tensor("wbd", [128, N_GROUPS * 64], F16, kind="ExternalInput")
    qwt = nc.dram_tensor("qwt", [128, N_PAIRS * NA], F16, kind="ExternalInput")
    qwto = nc.dram_tensor("qwto", [128, OWN_BLKS * NA], F16, kind="ExternalInput")
    gidx = nc.dram_tensor("gidx", [128, N_GROUPS * 8], I16, kind="ExternalInput")
    # 4 col-group partials stacked in the partition dim; host reduces them
    qp = nc.dram_tensor("qp", [128, B], F32, kind="ExternalOutput")

    with tile.TileContext(nc) as tc:
        with (
            tc.tile_pool(name="const", bufs=1) as const,
            tc.tile_pool(name="gbuf", bufs=4) as gbuf,
            tc.tile_pool(name="fbuf", bufs=3) as fbuf,
            tc.tile_pool(name="qout", bufs=1) as qout,
            tc.tile_pool(name="pre_ps", bufs=3, space="PSUM") as pre_ps,
            tc.tile_pool(name="qacc_ps", bufs=1, space="PSUM") as qacc_ps,
        ):
            gidx_sb = const.tile([128, N_GROUPS * 8], I16)
            wbd_sb = const.tile([128, N_GROUPS * 64], F16)
            qwt_sb = const.tile([128, N_PAIRS * NA], F16)
            qwto_sb = const.tile([128, OWN_BLKS * NA], F16)
            obt_sb = const.tile([128, OWN_BLKS * B], F16)
            # all const loads upfront: they complete for free during the
            # fixed ~11us GPSIMD LOAD_LIB window that gates the first gather
            nc.sync.dma_start(gidx_sb[:], gidx[:])
            nc.sync.dma_start(wbd_sb[:], wbd[:])
            nc.sync.dma_start(qwt_sb[:], qwt[:])
            nc.sync.dma_start(qwto_sb[:], qwto[:])
            nc.sync.dma_start(obt_sb[:], obst_own[:])

            qaccs = [
                qacc_ps.tile([128, HALF], F32, tag=f"qacc{h}", name=f"qacc{h}")
                for h in range(2)
            ]

            # 16 sub-chunks of 4 groups (512 gathered rows, 2 MB) each: small
            # drain granularity keeps the compute pipeline close behind the
            # drain stream and shortens the tail
            NSUB = 16
            SUBG = N_GROUPS // NSUB  # 4 groups per sub-chunk

            def gather(ci):
                s = ci * SUBG
                gt = gbuf.tile([128, SUBG, B], F16, tag="gt", name=f"gt_{s}")
                nc.gpsimd.dma_gather(
                    out_ap=gt[:],
                    in_ap=obst[:],
                    idxs_ap=gidx_sb[:, s * 8 : (s + SUBG) * 8],
                    num_idxs=SUBG * 128,
                    num_idxs_reg=SUBG * 128,
                    elem_size=B,
                )
                return gt

            PREF = 3  # gathers issued ahead of the consuming sub-chunk
            gts = {ci: gather(ci) for ci in range(PREF)}

            def gvf_pair(gt, feat, s, lp, bh, nb):
                kA = s + 2 * lp
                kB = kA + 1
                pre = pre_ps.tile(
                    [128, NB], F32, tag="pre", name=f"pre_{s}_{lp}_{bh}_{nb}"
                )
                col = bh * HALF + nb * NB
                nc.tensor.matmul(
                    pre[0:64, :],
                    wbd_sb[:, kA * 64 : (kA + 1) * 64],
                    gt[:, 2 * lp, col : col + NB],
                    start=True,
                    stop=True,
                    tile_position=(0, 0),
                )
                nc.tensor.matmul(
                    pre[64:128, :],
                    wbd_sb[:, kB * 64 : (kB + 1) * 64],
                    gt[:, 2 * lp + 1, col : col + NB],
                    start=True,
                    stop=True,
                    tile_position=(0, 64),
                )
                dst = feat[:, 2 * lp + bh, nb * NB : (nb + 1) * NB]
                if bh == 0:
                    nc.scalar.activation(
                        dst, pre[:], mybir.ActivationFunctionType.Relu
                    )
                else:
                    nc.vector.tensor_scalar_max(dst, pre[:], 0.0)

            def q_mm(feat, s, lp, bh, nb):
                P = s // 2 + lp
                j = P % 4
                nc.tensor.matmul(
                    qaccs[bh][32 * j : 32 * j + NA, nb * NB : (nb + 1) * NB],
                    qwt_sb[:, P * NA : (P + 1) * NA],
                    feat[:, 2 * lp + bh, nb * NB : (nb + 1) * NB],
                    start=(P < 4),
                    stop=(P >= N_PAIRS - 4),
                    tile_position=(0, 32 * j),
                )

            def q_out(bh):
                qsb = qout.tile([128, HALF], F32, tag=f"qsb{bh}", name=f"qsb{bh}")
                nc.vector.tensor_copy(qsb[:], qaccs[bh][:])
                nc.sync.dma_start(qp[:, bh * HALF : (bh + 1) * HALF], qsb[:])

            prev = None  # feat of the previous sub-chunk, Q-head deferred
            for ci in range(NSUB):
                s = ci * SUBG
                gt = gts.pop(ci)
                if ci + PREF < NSUB:
                    gts[ci + PREF] = gather(ci + PREF)
                # feat[:, 2*lp + bh, :] = relu of pair-tile features
                feat = fbuf.tile([128, SUBG, HALF], F16, tag="feat", name=f"feat_{s}")
                last = ci == NSUB - 1
                if not last:
                    for lp in range(SUBG // 2):
                        for bh in range(2):
                            for nb in range(2):
                                gvf_pair(gt, feat, s, lp, bh, nb)
                    if ci % 2 == 1:
                        # deferred Q-head for two sub-chunks: 4 consecutive
                        # pairs = col groups 0..3 issued back-to-back ->
                        # 4-way concurrent in the array
                        for bh in range(2):
                            for nb in range(2):
                                for lp in range(SUBG // 2):
                                    q_mm(prev[0], prev[1], lp, bh, nb)
                                for lp in range(SUBG // 2):
                                    q_mm(feat, s, lp, bh, nb)
                else:
                    # tail: finish batch-half 0 completely first so its
                    # output copy + DMA overlap half 1's compute
                    for bh in range(2):
                        for lp in range(SUBG // 2):
                            for nb in range(2):
                                gvf_pair(gt, feat, s, lp, bh, nb)
                        for nb in range(2):
                            for lp in range(SUBG // 2):
                                q_mm(prev[0], prev[1], lp, bh, nb)
                            for lp in range(SUBG // 2):
                                q_mm(feat, s, lp, bh, nb)
                        q_out(bh)
                prev = (feat, s)
                if ci == 3:
                    # raw-obs part of the Q head: one K-tile per col group
                    for bh in range(2):
                        for nb in range(2):
                            for j in range(OWN_BLKS):
                                o0 = j * B + bh * HALF + nb * NB
                                nc.tensor.matmul(
                                    qaccs[bh][
                                        32 * j : 32 * j + NA, nb * NB : (nb + 1) * NB
                                    ],
                                    qwto_sb[:, j * NA : (j + 1) * NA],
                                    obt_sb[:, o0 : o0 + NB],
                                    start=False,
                                    stop=False,
                                    tile_position=(0, 32 * j),
                                )

    nc.finalize()
    return nc


def _get_program():
    global _PROGRAM
    if _PROGRAM is None:
        _PROGRAM = _build_program()
    return _PROGRAM


def _stage_inputs(observation, gvf_W, q_W, gvf_input_idxs):
    """Host-side sharding/layout. Returns in_maps (list of dicts, one per core)."""
    obs = np.asarray(observation, dtype=np.float32)
    gw = np.asarray(gvf_W, dtype=np.float32)
    qw = np.asarray(q_W, dtype=np.float32)
    idx = np.asarray(gvf_input_idxs).astype(np.int64)

    obst = np.ascontiguousarray(obs.T.astype(np.float16))  # (OBS_DIM, B)

    in_maps = []
    for c in range(N_CORES):
        g0 = c * GPC

        # gather index plan: j = k*128 + p ; p = 16*a + i
        # idx_flat[j] = idx[g0 + 8k + a, i]
        k = np.arange(N_GROUPS)[:, None, None]  # group
        a = np.arange(8)[None, :, None]  # gvf within group
        i = np.arange(IPG)[None, None, :]  # input slot
        idx_flat = idx[g0 + 8 * k + a, i].reshape(N_GROUPS * 128)  # (8192,)
        # wrapped for dma_gather: per chunk of 1024 idxs, wrapped[p, s] = flat[s*16+p%16]
        per_call = CHUNK_GROUPS * 128
        gidx_h = np.zeros((128, N_GROUPS * 8), dtype=np.int16)
        for chunk in range(N_CHUNKS):
            fl = idx_flat[chunk * per_call : (chunk + 1) * per_call]
            wr = fl.reshape(per_call // 16, 16).T  # (16, S): wr[p, s] = fl[s*16+p]
            gidx_h[:, chunk * (per_call // 16) : (chunk + 1) * (per_call // 16)] = np.tile(
                wr, (8, 1)
            )

        # block-diagonal gvf weights: wbd[p, 64k + 8a + h] = gw[g0+8k+a, h, i]
        # with p = 16a + i
        wbd_h = np.zeros((128, N_GROUPS * 64), dtype=np.float16)
        kk = np.arange(N_GROUPS)[:, None, None, None]
        aa = np.arange(8)[None, :, None, None]
        hh = np.arange(HPG)[None, None, :, None]
        ii = np.arange(IPG)[None, None, None, :]
        vals = gw[g0 + 8 * kk + aa, hh, ii]  # (64, 8, 8, 16)
        p_idx = (16 * aa + ii).reshape(1, 8, 1, IPG)
        m_idx = (64 * kk + 8 * aa + hh).reshape(N_GROUPS, 8, HPG, 1)
        pf = np.broadcast_to(p_idx, vals.shape).reshape(-1)
        mf = np.broadcast_to(m_idx, vals.shape).reshape(-1)
        wbd_h[pf, mf] = vals.astype(np.float16).reshape(-1)

        # q-head weights for gvf features: pair tile P covers feat rows
        # pp in [0,128): k = 2P + pp//64, m = pp%64, gvf = g0+8k+m//8, h = m%8
        P = np.arange(N_PAIRS)[None, :]
        pp = np.arange(128)[:, None]
        kq = 2 * P + pp // 64
        m = pp % 64
        col = OBS_DIM + (g0 + 8 * kq + m // 8) * HPG + (m % 8)  # (128, 32)
        qwt_h = (
            qw[:, col].transpose(1, 2, 0).reshape(128, N_PAIRS * NA)
        ).astype(np.float16)

        # q-head weights for this core's raw-obs block
        f0 = c * (OBS_DIM // N_CORES)
        colo = f0 + np.arange(OWN_BLKS)[None, :] * 128 + np.arange(128)[:, None]
        qwto_h = (
            qw[:, colo].transpose(1, 2, 0).reshape(128, OWN_BLKS * NA)
        ).astype(np.float16)

        # (128, 4*B): col block ob = obsT rows [f0 + ob*128 .. +128)
        obst_own_h = np.ascontiguousarray(
            obst[f0 : f0 + OWN_BLKS * 128, :]
            .reshape(OWN_BLKS, 128, B)
            .transpose(1, 0, 2)
            .reshape(128, OWN_BLKS * B)
        )

        in_maps.append(
            {
                "obst": obst,
                "obst_own": obst_own_h,
                "wbd": wbd_h,
                "qwt": np.ascontiguousarray(qwt_h),
                "qwto": qwto_h,
                "gidx": gidx_h,
            }
        )
    return in_maps


def kernel(observation, gvf_W, q_W, gvf_input_idxs, _trace=False):
    nc = _get_program()
    in_maps = _stage_inputs(observation, gvf_W, q_W, gvf_input_idxs)
    res = run_bass_kernel_spmd(nc, in_maps, list(range(N_CORES)), trace=_trace)
    q = np.zeros((NA, B), dtype=np.float32)
    for c in range(N_CORES):
        r = res.results[c]["qp"]  # (128, B): 4 col-group partials
        for j in range(4):
            q += r[32 * j : 32 * j + NA, :]
    out = np.ascontiguousarray(q.T, dtype=np.float32)
    if _trace:
        kernel.last_exec_time_ns = res.exec_time_ns
    return out


# revision 14
# speedup vs baseline: 1.1057x; 1.1057x over previous
"""Trainium2 Bass kernel for nn_Nibbler_70755291234540 (gnn_message_passing).

q = concat(obs, relu(per-gvf tiny nets(gathered obs))) @ q_W.T

Strategy (8 NeuronCores, SPMD single program):
  - Shard the 4096 GVFs across cores (512/core); every core sees the full
    batch and produces a partial Q; host sums the partials.
  - Host pre-transposes obs -> obsT (4096, 2048) fp16 in DRAM. The per-GVF
    input gather becomes a row gather out of obsT done by dma_gather
    (GPSIMD SWDGE) straight into partition-major SBUF tiles: 128 gathered
    rows = one group of 8 GVFs x 16 inputs; 8 groups (4 MB) per call, 8
    calls pipelined back-to-back at ~HBM line rate.
  - Per pair of groups: col-tiled (K=128, M=64)x2 fp16 matmuls run both
    groups concurrently in the 128x128 array; relu+cast eviction of the
    full 128-partition PSUM tile on ACT (half 0) / DVE (half 1).
  - Q head: 36 K-tiles (32 gvf-feature pairs + 4 raw-obs blocks) col-tiled
    4-wide at M=18 into one 128-partition PSUM accumulator per batch half;
    col groups are summed on the host together with the cross-core sum.
"""

import sys
import types

import numpy as np

# ---- problem constants (hardcoded; kernel.py must be self-contained) ----
B = 2048
OBS_DIM = 4096
N_GVFS = 4096
IPG = 16  # inputs per gvf
HPG = 8  # hidden per gvf
NA = 18  # actions
N_CORES = 8
GPC = N_GVFS // N_CORES  # 512 gvfs per core
N_GROUPS = GPC // 8  # 64 groups of 8 gvfs -> 128 gathered rows each
N_PAIRS = N_GROUPS // 2  # 32 pair tiles (128 feature rows each)
HALF = B // 2  # batch slab (PSUM-sized)
NB = 512  # matmul moving-operand chunk
CHUNK_GROUPS = 8  # gvf groups per dma_gather call (1024 rows, full batch)
N_CHUNKS = N_GROUPS // CHUNK_GROUPS  # 8
OWN_BLKS = (OBS_DIM // N_CORES) // 128  # 4 obs-feature blocks per core


def _install_axon_profile_hook():
    """bass_utils trace=True under axon needs antenv.axon_hooks; shim it."""
    try:
        import antenv
    except ImportError:
        return
    if "antenv.axon_hooks" in sys.modules:
        return
    hooks = types.ModuleType("antenv.axon_hooks")
    hooks._hook = None

    def set_axon_ntff_profile_hook(h):
        hooks._hook = h

    def get_axon_ntff_profile_hook():
        return hooks._hook

    hooks.set_axon_ntff_profile_hook = set_axon_ntff_profile_hook
    hooks.get_axon_ntff_profile_hook = get_axon_ntff_profile_hook
    sys.modules["antenv.axon_hooks"] = hooks
    antenv.axon_hooks = hooks
    try:
        from trn_agent_boot.trn_boot import _ntff_profile_via_ctypes

        hook = _ntff_profile_via_ctypes("/opt/axon/libaxon_pjrt.so")
        if hook is not None:
            set_axon_ntff_profile_hook(hook)
    except Exception:
        pass


_install_axon_profile_hook()

import concourse.bacc as bacc
import concourse.bass as bass
import concourse.mybir as mybir
import concourse.tile as tile
from concourse.bass_utils import run_bass_kernel_spmd

F16 = mybir.dt.float16
F32 = mybir.dt.float32
I16 = mybir.dt.int16

_PROGRAM = None


def _build_program():
    nc = bacc.Bacc(None, target_bir_lowering=False, debug=False, num_devices=N_CORES)

    obst = nc.dram_tensor("obst", [OBS_DIM, B], F16, kind="ExternalInput")
    # own obs block, pre-arranged (128, 4*B): col block ob holds obsT rows
    # [own0 + ob*128 + p]
    obst_own = nc.dram_tensor("obst_own", [128, OWN_BLKS * B], F16, kind="ExternalInput")
    wbd = nc.dram_tensor("wbd", [128, N_GROUPS * 64], F16, kind="ExternalInput")
    qwt = nc.dram_tensor("qwt", [128, N_PAIRS * NA], F16, kind="ExternalInput")
    qwto = nc.dram_tensor("qwto", [128, OWN_BLKS * NA], F16, kind="ExternalInput")
    gidx = nc.dram_tensor("gidx", [128, N_GROUPS * 8], I16, kind="ExternalInput")
    # 4 col-group partials stacked in the partition dim; host reduces them
    qp = nc.dram_tensor("qp", [128, B], F32, kind="ExternalOutput")

    with tile.TileContext(nc) as tc:
        with (
            tc.tile_pool(name="const", bufs=1) as const,
            tc.tile_pool(name="gbuf", bufs=4) as gbuf,
            tc.tile_pool(name="fbuf", bufs=2) as fbuf,
            tc.tile_pool(name="qout", bufs=1) as qout,
            tc.tile_pool(name="pre_ps", bufs=3, space="PSUM") as pre_ps,
            tc.tile_pool(name="qacc_ps", bufs=1, space="PSUM") as qacc_ps,
        ):
            gidx_sb = const.tile([128, N_GROUPS * 8], I16)
            wbd_sb = const.tile([128, N_GROUPS * 64], F16)
            qwt_sb = const.tile([128, N_PAIRS * NA], F16)
            qwto_sb = const.tile([128, OWN_BLKS * NA], F16)
            obt_sb = const.tile([128, OWN_BLKS * B], F16)
            # all const loads upfront: they complete for free during the
            # fixed ~11us GPSIMD LOAD_LIB window that gates the first gather
            nc.sync.dma_start(gidx_sb[:], gidx[:])
            nc.sync.dma_start(wbd_sb[:], wbd[:])
            nc.sync.dma_start(qwt_sb[:], qwt[:])
            nc.sync.dma_start(qwto_sb[:], qwto[:])
            nc.sync.dma_start(obt_sb[:], obst_own[:])

            qaccs = [
                qacc_ps.tile([128, HALF], F32, tag=f"qacc{h}", name=f"qacc{h}")
                for h in range(2)
            ]

            # 16 sub-chunks of 4 groups (512 gathered rows, 2 MB) each: small
            # drain granularity keeps the compute pipeline close behind the
            # drain stream and shortens the tail
            NSUB = 16
            SUBG = N_GROUPS // NSUB  # 4 groups per sub-chunk

            def gather(ci):
                s = ci * SUBG
                gt = gbuf.tile([128, SUBG, B], F16, tag="gt", name=f"gt_{s}")
                nc.gpsimd.dma_gather(
                    out_ap=gt[:],
                    in_ap=obst[:],
                    idxs_ap=gidx_sb[:, s * 8 : (s + SUBG) * 8],
                    num_idxs=SUBG * 128,
                    num_idxs_reg=SUBG * 128,
                    elem_size=B,
                )
                return gt

            PREF = 3  # gathers issued ahead of the consuming sub-chunk
            gts = {ci: gather(ci) for ci in range(PREF)}

            def gvf_pair(gt, feat, s, lp, bh, nb):
                kA = s + 2 * lp
                kB = kA + 1
                pre = pre_ps.tile(
                    [128, NB], F32, tag="pre", name=f"pre_{s}_{lp}_{bh}_{nb}"
                )
                col = bh * HALF + nb * NB
                nc.tensor.matmul(
                    pre[0:64, :],
                    wbd_sb[:, kA * 64 : (kA + 1) * 64],
                    gt[:, 2 * lp, col : col + NB],
                    start=True,
                    stop=True,
                    tile_position=(0, 0),
                )
                nc.tensor.matmul(
                    pre[64:128, :],
                    wbd_sb[:, kB * 64 : (kB + 1) * 64],
                    gt[:, 2 * lp + 1, col : col + NB],
                    start=True,
                    stop=True,
                    tile_position=(0, 64),
                )
                dst = feat[:, 2 * lp + bh, nb * NB : (nb + 1) * NB]
                if bh == 0:
                    nc.scalar.activation(
                        dst, pre[:], mybir.ActivationFunctionType.Relu
                    )
                else:
                    nc.vector.tensor_scalar_max(dst, pre[:], 0.0)

            def q_mm(feat, s, lp, bh, nb):
                P = s // 2 + lp
                j = P % 4
                nc.tensor.matmul(
                    qaccs[bh][32 * j : 32 * j + NA, nb * NB : (nb + 1) * NB],
                    qwt_sb[:, P * NA : (P + 1) * NA],
                    feat[:, 2 * lp + bh, nb * NB : (nb + 1) * NB],
                    start=(P < 4),
                    stop=(P >= N_PAIRS - 4),
                    tile_position=(0, 32 * j),
                )

            def q_out(bh):
                qsb = qout.tile([128, HALF], F32, tag=f"qsb{bh}", name=f"qsb{bh}")
                nc.vector.tensor_copy(qsb[:], qaccs[bh][:])
                nc.sync.dma_start(qp[:, bh * HALF : (bh + 1) * HALF], qsb[:])

            for ci in range(NSUB):
                s = ci * SUBG
                gt = gts.pop(ci)
                if ci + PREF < NSUB:
                    gts[ci + PREF] = gather(ci + PREF)
                # feat[:, 2*lp + bh, :] = relu of pair-tile features
                feat = fbuf.tile([128, SUBG, HALF], F16, tag="feat", name=f"feat_{s}")
                last = ci == NSUB - 1
                if not last:
                    # bh-major: half 0's evictions complete while the PE
                    # runs half 1's GVF matmuls, so the Q matmuls never
                    # stall on eviction semaphores
                    for bh in range(2):
                        for lp in range(SUBG // 2):
                            for nb in range(2):
                                gvf_pair(gt, feat, s, lp, bh, nb)
                    for bh in range(2):
                        for nb in range(2):
                            for lp in range(SUBG // 2):
                                q_mm(feat, s, lp, bh, nb)
                else:
                    # tail: finish batch-half 0 completely first so its
                    # output copy + DMA overlap half 1's compute
                    for bh in range(2):
                        for lp in range(SUBG // 2):
                            for nb in range(2):
                                gvf_pair(gt, feat, s, lp, bh, nb)
                        for nb in range(2):
                            for lp in range(SUBG // 2):
                                q_mm(feat, s, lp, bh, nb)
                        q_out(bh)
                if ci == 3:
                    # raw-obs part of the Q head: one K-tile per col group
                    for bh in range(2):
                        for nb in range(2):
                            for j in range(OWN_BLKS):
                                o0 = j * B + bh * HALF + nb * NB
                                nc.tensor.matmul(
                                    qaccs[bh][
                                        32 * j : 32 * j + NA, nb * NB : (nb + 1) * NB
                                    ],
                                    qwto_sb[:, j * NA : (j + 1) * NA],
                                    obt_sb[:, o0 : o0 + NB],
                                    start=False,
                                    stop=False,
                                    tile_position=(0, 32 * j),
                                )

    nc.finalize()
    return nc


def _get_program():
    global _PROGRAM
    if _PROGRAM is None:
        _PROGRAM = _build_program()
    return _PROGRAM


def _stage_inputs(observation, gvf_W, q_W, gvf_input_idxs):
    """Host-side sharding/layout. Returns in_maps (list of dicts, one per core)."""
    obs = np.asarray(observation, dtype=np.float32)
    gw = np.asarray(gvf_W, dtype=np.float32)
    qw = np.asarray(q_W, dtype=np.float32)
    idx = np.asarray(gvf_input_idxs).astype(np.int64)

    obst = np.ascontiguousarray(obs.T.astype(np.float16))  # (OBS_DIM, B)

    in_maps = []
    for c in range(N_CORES):
        g0 = c * GPC

        # gather index plan: j = k*128 + p ; p = 16*a + i
        # idx_flat[j] = idx[g0 + 8k + a, i]
        k = np.arange(N_GROUPS)[:, None, None]  # group
        a = np.arange(8)[None, :, None]  # gvf within group
        i = np.arange(IPG)[None, None, :]  # input slot
        idx_flat = idx[g0 + 8 * k + a, i].reshape(N_GROUPS * 128)  # (8192,)
        # wrapped for dma_gather: per chunk of 1024 idxs, wrapped[p, s] = flat[s*16+p%16]
        per_call = CHUNK_GROUPS * 128
        gidx_h = np.zeros((128, N_GROUPS * 8), dtype=np.int16)
        for chunk in range(N_CHUNKS):
            fl = idx_flat[chunk * per_call : (chunk + 1) * per_call]
            wr = fl.reshape(per_call // 16, 16).T  # (16, S): wr[p, s] = fl[s*16+p]
            gidx_h[:, chunk * (per_call // 16) : (chunk + 1) * (per_call // 16)] = np.tile(
                wr, (8, 1)
            )

        # block-diagonal gvf weights: wbd[p, 64k + 8a + h] = gw[g0+8k+a, h, i]
        # with p = 16a + i
        wbd_h = np.zeros((128, N_GROUPS * 64), dtype=np.float16)
        kk = np.arange(N_GROUPS)[:, None, None, None]
        aa = np.arange(8)[None, :, None, None]
        hh = np.arange(HPG)[None, None, :, None]
        ii = np.arange(IPG)[None, None, None, :]
        vals = gw[g0 + 8 * kk + aa, hh, ii]  # (64, 8, 8, 16)
        p_idx = (16 * aa + ii).reshape(1, 8, 1, IPG)
        m_idx = (64 * kk + 8 * aa + hh).reshape(N_GROUPS, 8, HPG, 1)
        pf = np.broadcast_to(p_idx, vals.shape).reshape(-1)
        mf = np.broadcast_to(m_idx, vals.shape).reshape(-1)
        wbd_h[pf, mf] = vals.astype(np.float16).reshape(-1)

        # q-head weights for gvf features: pair tile P covers feat rows
        # pp in [0,128): k = 2P + pp//64, m = pp%64, gvf = g0+8k+m//8, h = m%8
        P = np.arange(N_PAIRS)[None, :]
        pp = np.arange(128)[:, None]
        kq = 2 * P + pp // 64
        m = pp % 64
        col = OBS_DIM + (g0 + 8 * kq + m // 8) * HPG + (m % 8)  # (128, 32)
        qwt_h = (
            qw[:, col].transpose(1, 2, 0).reshape(128, N_PAIRS * NA)
        ).astype(np.float16)

        # q-head weights for this core's raw-obs block
        f0 = c * (OBS_DIM // N_CORES)
        colo = f0 + np.arange(OWN_BLKS)[None, :] * 128 + np.arange(128)[:, None]
        qwto_h = (
            qw[:, colo].transpose(1, 2, 0).reshape(128, OWN_BLKS * NA)
        ).astype(np.float16)

        # (128, 4*B): col block ob = obsT rows [f0 + ob*128 .. +128)
        obst_own_h = np.ascontiguousarray(
            obst[f0 : f0 + OWN_BLKS * 128, :]
            .reshape(OWN_BLKS, 128, B)
            .transpose(1, 0, 2)
            .reshape(128, OWN_BLKS * B)
        )

        in_maps.append(
            {
                "obst": obst,
                "obst_own": obst_own_h,
                "wbd": wbd_h,
                "qwt": np.ascontiguousarray(qwt_h),
                "qwto": qwto_h,
                "gidx": gidx_h,
            }
        )
    return in_maps


def kernel(observation, gvf_W, q_W, gvf_input_idxs, _trace=False):
    nc = _get_program()
    in_maps = _stage_inputs(observation, gvf_W, q_W, gvf_input_idxs)
    res = run_bass_kernel_spmd(nc, in_maps, list(range(N_CORES)), trace=_trace)
    q = np.zeros((NA, B), dtype=np.float32)
    for c in range(N_CORES):
        r = res.results[c]["qp"]  # (128, B): 4 col-group partials
        for j in range(4):
            q += r[32 * j : 32 * j + NA, :]
    out = np.ascontiguousarray(q.T, dtype=np.float32)
    if _trace:
        kernel.last_exec_time_ns = res.exec_time_ns
    return out
